# revision 1
# baseline (speedup 1.0000x reference)
"""Trainium2 Bass kernel for NnqlmCnnBasedLstm.

Math (per batch item, per input sequence q/a):
  xe = embed[idx]                      (L, D)       D = 128
  dens_t = outer(xe_t, xe_t)/(|xe_t|^2 + 1e-4)     (D, D), symmetric
  2-layer ConvLSTM over L=40 steps; each gate g:
    pre_g = conv2d([xt; h], W_g, stride=(2,1), pad=(1,1)) + b_g  on (2D, D) -> (D, D)
  c = sig(f)*c + sig(i)*tanh(cc); h = sig(o)*tanh(c)
  out = max_t h2_t  -> flatten -> concat(q,a) -> linear(2) -> log_softmax

Device strategy (8 cores, data parallel over B=32 -> 4 items/core, each with a
q-chain and an a-chain = 8 chains/core):
  * State kept TRANSPOSED: tiles are (w partitions, j free).  The density is
    symmetric so layer-1 inputs need no transpose.
  * conv: out_T[w, j] = sum_{dh,dw} W[dh,dw] * inp_T[w-1+dw, 2j-1+dh].
    For each dh this is a 3-diagonal Toeplitz band matrix (over w) applied via
    the TensorEngine, with the (2j-1+dh) selection expressed as a stride-2
    free-axis access pattern on the moving operand.  4 gates x 4 dh matmuls
    accumulate in PSUM; all 8 chains batched in the moving free dim.
  * sigmoid/tanh (+conv bias) on ScalarE reading PSUM; cell updates on VectorE;
    densities as rank-1 (K=1) outer-product matmuls on the TensorEngine.
  * Embedding gather, final linear + log_softmax on host (tiny).
"""

import os
import sys

import numpy as np

for _p in ("/opt/trn_rl_repo", "/root/.axon_site/_ro/trn_rl_repo"):
    if os.path.isdir(_p) and _p not in sys.path:
        sys.path.insert(0, _p)

B, L, D, V, NL = 32, 40, 128, 32000, 2
NCORES = 8
CH = 8            # chains per core: 4 batch items x {q, a}
SEG = 2 * D + 2   # per-chain column span in the input tile: [0]=0, [1..128]=x, [129..256]=h, [257]=0
NF = CH * SEG
NV = L * CH       # 320 embedding vectors per core
NVP = 384         # padded to a multiple of 128

_CACHE = {}


def _build_nc(L=L):
    import concourse.bass as bass
    import concourse.bacc as bacc
    import concourse.mybir as mybir
    from concourse import tile

    f32 = mybir.dt.float32
    AF = mybir.ActivationFunctionType
    ALU = mybir.AluOpType

    nc = bacc.Bacc(None, target_bir_lowering=False)

    xey_d = nc.dram_tensor("xey", (L, 1, CH * D), f32, kind="ExternalInput")
    st_d = nc.dram_tensor("st", (NL * 4 * 4, D, D), f32, kind="ExternalInput")
    bias_d = nc.dram_tensor("bias", (D, NL * 4), f32, kind="ExternalInput")
    out_d = nc.dram_tensor("mp_out", (D, CH * D), f32, kind="ExternalOutput")

    with tile.TileContext(nc) as tc:
        with (
            tc.tile_pool(name="const", bufs=1) as constp,
            tc.tile_pool(name="state", bufs=1) as statep,
            tc.tile_pool(name="inp", bufs=2) as inpp,
            tc.tile_pool(name="gate", bufs=2) as gatep,
            tc.tile_pool(name="psum", bufs=1, space="PSUM") as psump,
        ):
            # ---- constants ----
            stT = constp.tile([D, NL * 4 * 4 * D], f32, tag="stT")
            for i in range(NL * 4 * 4):
                nc.sync.dma_start(stT[:, i * D:(i + 1) * D], st_d[i])

            bias = constp.tile([D, NL * 4], f32, tag="bias")
            nc.sync.dma_start(bias[:], bias_d[:])

            # ---- persistent state ----
            c_l = [statep.tile([D, CH * D], f32, tag=f"c{l}", name=f"c{l}") for l in range(NL)]
            mp = statep.tile([D, CH * D], f32, tag="mp")
            for l in range(NL):
                nc.vector.memset(c_l[l][:], 0.0)
            nc.vector.memset(mp[:], -1e30)

            def seg3(t):  # (p, s, c) view of an input tile
                return t[:].rearrange("p (s c) -> p s c", s=CH)

            def seg4(t):  # (p, s, c2, two) parity view for stride-2 j access
                return t[:].rearrange("p (s c two) -> p s c two", s=CH, two=2)

            def new_inp(tag):
                t = inpp.tile([D, NF], f32, tag=tag, name=tag)
                # zero the pad columns (0 and 257 of each chain segment)
                v = t[:].rearrange("p (s c) -> p s c", s=CH)
                nc.gpsimd.memset(v[:, :, 0:1], 0.0)
                nc.gpsimd.memset(v[:, :, SEG - 1:SEG], 0.0)
                return t

            def outers(t_next, dst_tile):
                """Rank-1 matmuls: densities for step t_next -> x-part of dst_tile."""
                stage = gatep.tile([1, CH * D], f32, tag="xstage", name="xstage")
                nc.sync.dma_start(stage[:], xey_d[t_next])
                po = psump.tile([D, CH * D], f32, tag="pf", name="po")
                for s in range(CH):
                    vec = stage[0:1, s * D:(s + 1) * D]
                    nc.tensor.matmul(
                        po[:, s * D:(s + 1) * D],
                        vec, vec,
                        start=True, stop=True,
                    )
                v3 = seg3(dst_tile)
                for hf in range(2):
                    nc.scalar.activation(v3[:, hf * 4:(hf + 1) * 4, 1:1 + D],
                                         po[:, hf * 512:(hf + 1) * 512], AF.Copy)

            cur = [None, None]
            cur[0] = new_inp("inp0")
            cur[1] = new_inp("inp1")
            nc.gpsimd.memset(seg3(cur[0])[:, :, 129:129 + D], 0.0)   # h1_{-1} = 0
            nc.gpsimd.memset(seg3(cur[1])[:, :, 129:129 + D], 0.0)   # h2_{-1} = 0
            outers(0, cur[0])

            GTAG = ["pf", "pi", "po", "pc"]
            for t in range(L):
                nxt = [None, None]
                nxt[0] = new_inp("inp0") if t + 1 < L else None
                nxt[1] = new_inp("inp1") if t + 1 < L else None

                for l in range(NL):
                    inp = cur[l]
                    i4 = seg4(inp)
                    # --- gate pre-activations: 4 gates x 4 dh band matmuls ---
                    ps = [psump.tile([D, CH * D], f32, tag=GTAG[g], name=GTAG[g]) for g in range(4)]
                    for g in range(4):
                        for half in range(2):
                            for dh in range(4):
                                idx = (l * 4 + g) * 4 + dh
                                rhs = i4[:, half * 4:(half + 1) * 4,
                                         dh // 2: dh // 2 + D, dh % 2]
                                nc.tensor.matmul(
                                    ps[g][:, half * 512:(half + 1) * 512],
                                    stT[:, idx * D:(idx + 1) * D],
                                    rhs,
                                    start=(dh == 0), stop=(dh == 3),
                                )
                    # --- activations (bias folded in) ---
                    fg = gatep.tile([D, CH * D], f32, tag="fg")
                    ig = gatep.tile([D, CH * D], f32, tag="ig")
                    og = gatep.tile([D, CH * D], f32, tag="og")
                    cs = gatep.tile([D, CH * D], f32, tag="cs")
                    for g, dst in enumerate((fg, ig, og)):
                        nc.scalar.activation(dst[:], ps[g][:], AF.Sigmoid,
                                             bias=bias[:, l * 4 + g: l * 4 + g + 1])
                    nc.scalar.activation(cs[:], ps[3][:], AF.Tanh,
                                         bias=bias[:, l * 4 + 3: l * 4 + 4])
                    # --- cell update ---
                    t1 = gatep.tile([D, CH * D], f32, tag="t1")
                    t2 = gatep.tile([D, CH * D], f32, tag="t2")
                    nc.vector.tensor_mul(t1[:], fg[:], c_l[l][:])
                    nc.vector.tensor_mul(t2[:], ig[:], cs[:])
                    nc.vector.tensor_add(c_l[l][:], t1[:], t2[:])
                    th = gatep.tile([D, CH * D], f32, tag="th")
                    nc.scalar.activation(th[:], c_l[l][:], AF.Tanh)
                    # --- h = og * tanh(c): route to consumers ---
                    if l == 0:
                        # h1_t -> x-part of layer-2 input (this step)
                        nc.vector.tensor_mul(seg3(cur[1])[:, :, 1:1 + D], og[:], th[:])
                        if nxt[0] is not None:
                            # copy h1_t -> h-part of next layer-1 input
                            nc.gpsimd.tensor_copy(
                                seg3(nxt[0])[:, :, 129:129 + D],
                                seg3(cur[1])[:, :, 1:1 + D],
                            )
                    else:
                        if nxt[1] is not None:
                            h2dst = seg3(nxt[1])[:, :, 129:129 + D]
                            nc.vector.tensor_mul(h2dst, og[:], th[:])
                            nc.vector.tensor_tensor(mp[:], mp[:], h2dst, op=ALU.max)
                        else:
                            h2 = gatep.tile([D, CH * D], f32, tag="h2last")
                            nc.vector.tensor_mul(h2[:], og[:], th[:])
                            nc.vector.tensor_tensor(mp[:], mp[:], h2[:], op=ALU.max)

                if nxt[0] is not None:
                    outers(t + 1, nxt[0])
                cur = nxt

            nc.sync.dma_start(out_d[:], mp[:])

    nc.compile()
    return nc


def _prep_core_inputs(xe_y, st, bias_arr, core):
    """xe_y: (B, 2, L, D) sqrt-normalized embeddings (axis1: 0=q, 1=a)."""
    sl = slice(4 * core, 4 * core + 4)
    # chains: s=0..3 -> q items, s=4..7 -> a items
    ch = np.concatenate([xe_y[sl, 0], xe_y[sl, 1]], axis=0)    # (8, L, D)
    xey = np.ascontiguousarray(ch.transpose(1, 0, 2)).reshape(L, 1, CH * D)
    return {"xey": xey, "st": st, "bias": bias_arr}


def kernel(q, a, embed, conv_w, conv_b, lin_w, lin_b):
    from concourse import bass_utils

    q = np.asarray(q); a = np.asarray(a)
    embed = np.asarray(embed, np.float32)
    conv_w = np.asarray(conv_w, np.float32)
    conv_b = np.asarray(conv_b, np.float32)
    lin_w = np.asarray(lin_w, np.float32)
    lin_b = np.asarray(lin_b, np.float32)

    # host: embedding gather + density normalization factors
    idx = np.stack([q, a], axis=1).astype(np.int64)            # (B, 2, L)
    xe = embed[idx].astype(np.float64)                         # (B, 2, L, D)
    dot = np.sum(xe * xe, axis=-1, keepdims=True) + 1e-4
    xe_y = (xe / np.sqrt(dot)).astype(np.float32)

    # host: Toeplitz band stationaries  lhsT[(l,g,dh)] = B^T,
    # B[w, w'] = W[dh, w'-w+1]  (3 diagonals)
    st = np.zeros((NL * 4 * 4, D, D), np.float32)
    for l in range(NL):
        for g in range(4):
            W = conv_w[l, g, 0, 0]                             # (4, 3)
            for dh in range(4):
                Bm = sum(W[dh, dw] * np.eye(D, k=dw - 1) for dw in range(3))
                st[(l * 4 + g) * 4 + dh] = Bm.T.astype(np.float32)
    bias_arr = np.tile(conv_b.reshape(1, -1), (D, 1)).astype(np.float32)

    if "nc" not in _CACHE:
        _CACHE["nc"] = _build_nc()
    nc = _CACHE["nc"]

    in_maps = [_prep_core_inputs(xe_y, st, bias_arr, i) for i in range(NCORES)]
    _CACHE["in_maps"] = in_maps
    res = bass_utils.run_bass_kernel_spmd(nc, in_maps, core_ids=list(range(NCORES)))

    # host: unshard + final linear + log_softmax
    q_p = np.zeros((B, D * D), np.float32)
    a_p = np.zeros((B, D * D), np.float32)
    for i in range(NCORES):
        out = res.results[i]["mp_out"]                         # (D w, CH*D)
        for s in range(CH):
            mp_T = out[:, s * D:(s + 1) * D]                   # (w, j)
            flat = np.ascontiguousarray(mp_T.T).reshape(-1)    # j-major
            if s < 4:
                q_p[4 * i + s] = flat
            else:
                a_p[4 * i + s - 4] = flat
    qa = np.concatenate([q_p, a_p], axis=1)
    score = qa @ lin_w.T + lin_b
    m = score.max(axis=1, keepdims=True)
    ls = score - m
    lse = np.log(np.exp(ls).sum(axis=1, keepdims=True))
    return (ls - lse).astype(np.float32)



# revision 18
# speedup vs baseline: 1.8578x; 1.8578x over previous
"""Trainium2 Bass kernel for NnqlmCnnBasedLstm.

Math (per batch item, per input sequence q/a):
  xe = embed[idx]                      (L, D)       D = 128
  dens_t = outer(xe_t, xe_t)/(|xe_t|^2 + 1e-4)     (D, D), symmetric
  2-layer ConvLSTM over L=40 steps; each gate g:
    pre_g = conv2d([xt; h], W_g, stride=(2,1), pad=(1,1)) + b_g  on (2D, D) -> (D, D)
  c = sig(f)*c + sig(i)*tanh(cc); h = sig(o)*tanh(c)
  out = max_t h2_t  -> flatten -> concat(q,a) -> linear(2) -> log_softmax

Device strategy (8 cores, data parallel over B=32 -> 4 items/core, each with a
q-chain and an a-chain = 8 chains/core):
  * State kept TRANSPOSED: tiles are (w partitions, j free).  The density is
    symmetric so layer-1 inputs need no transpose.
  * conv: out_T[w, j] = sum_{dh,dw} W[dh,dw] * inp_T[w-1+dw, 2j-1+dh].
    For each dh this is a 3-diagonal Toeplitz band matrix (over w) applied via
    the TensorEngine, with the (2j-1+dh) selection expressed as a stride-2
    free-axis access pattern on the moving operand.  4 gates x 4 dh matmuls
    accumulate in PSUM; all 8 chains batched in the moving free dim.
  * sigmoid/tanh (+conv bias) on ScalarE reading PSUM; cell updates on VectorE;
    densities as rank-1 (K=1) outer-product matmuls on the TensorEngine.
  * Embedding gather, final linear + log_softmax on host (tiny).
"""

import os
import sys

import numpy as np

for _p in ("/opt/trn_rl_repo", "/root/.axon_site/_ro/trn_rl_repo"):
    if os.path.isdir(_p) and _p not in sys.path:
        sys.path.insert(0, _p)

B, L, D, V, NL = 32, 40, 128, 32000, 2
NCORES = 8
CH = 8            # chains per core: 4 batch items x {q, a}
SEG = 2 * D + 2   # per-chain column span in the input tile: [0]=0, [1..128]=x, [129..256]=h, [257]=0
NF = CH * SEG
NV = L * CH       # 320 embedding vectors per core
NVP = 384         # padded to a multiple of 128

_CACHE = {}


def _build_nc(L=L):
    import concourse.bass as bass
    import concourse.bacc as bacc
    import concourse.mybir as mybir
    from concourse import tile

    f32 = mybir.dt.float32
    f32r = mybir.dt.float32r
    AF = mybir.ActivationFunctionType
    ALU = mybir.AluOpType

    nc = bacc.Bacc(None, target_bir_lowering=False)

    xey_d = nc.dram_tensor("xey", (L, 1, CH * D), f32r, kind="ExternalInput")
    st_d = nc.dram_tensor("st", (NL * 4 * 4, D, D), f32r, kind="ExternalInput")
    zpad_d = nc.dram_tensor("zpad", (D, CH * D), f32r, kind="ExternalInput")
    bias_d = nc.dram_tensor("bias", (D, NL * 4), f32, kind="ExternalInput")
    out_d = nc.dram_tensor("mp_out", (D, CH * D), f32, kind="ExternalOutput")

    with tile.TileContext(nc) as tc:
        with (
            tc.tile_pool(name="const", bufs=1) as constp,
            tc.tile_pool(name="state", bufs=1) as statep,
            tc.tile_pool(name="inp", bufs=2) as inpp,
            tc.tile_pool(name="gate", bufs=2) as gatep,
            tc.tile_pool(name="psum", bufs=1, space="PSUM") as psump,
        ):
            # ---- constants ----
            stT = constp.tile([D, NL * 4 * 4 * D], f32r, tag="stT")
            for i in range(NL * 4 * 4):
                nc.sync.dma_start(stT[:, i * D:(i + 1) * D], st_d[i])

            bias = constp.tile([D, NL * 4], f32, tag="bias")
            nc.sync.dma_start(bias[:], bias_d[:])

            # ---- persistent state ----
            c_l = [statep.tile([D, CH * D], f32, tag=f"c{l}", name=f"c{l}") for l in range(NL)]
            mp = statep.tile([D, CH * D], f32, tag="mp")
            for l in range(NL):
                nc.vector.memset(c_l[l][:], 0.0)
            nc.vector.memset(mp[:], -1e30)

            def seg3(t):  # (p, s, c) view of an input tile
                return t[:].rearrange("p (s c) -> p s c", s=CH)

            def seg4(t):  # (p, s, c2, two) parity view for stride-2 j access
                return t[:].rearrange("p (s c two) -> p s c two", s=CH, two=2)

            def new_inp(tag, zero_pads):
                t = inpp.tile([D, NF], f32r, tag=tag, name=tag)
                if zero_pads:
                    # zero the pad columns (0 and 257 of each chain segment);
                    # they are never overwritten, so only the first
                    # acquisition of each physical pool buffer needs this
                    v = t[:].rearrange("p (s c) -> p s c", s=CH)
                    nc.sync.dma_start(v[:, :, 0:1], zpad_d[:, 0:CH])
                    nc.sync.dma_start(v[:, :, SEG - 1:SEG], zpad_d[:, 0:CH])
                return t

            def outers(t_next, dst_tile):
                """Rank-1 matmuls: densities for step t_next -> x-part of dst_tile."""
                stage = gatep.tile([1, CH * D], f32r, tag="xstage", name="xstage")
                nc.sync.dma_start(stage[:], xey_d[t_next])
                po = psump.tile([D, CH * D], f32, tag="pf", name="po")
                for s in range(CH):
                    vec = stage[0:1, s * D:(s + 1) * D]
                    nc.tensor.matmul(
                        po[:, s * D:(s + 1) * D],
                        vec, vec,
                        start=True, stop=True,
                    )
                v3 = seg3(dst_tile)
                for hf in range(2):
                    nc.scalar.activation(v3[:, hf * 4:(hf + 1) * 4, 1:1 + D],
                                         po[:, hf * 512:(hf + 1) * 512], AF.Copy)

            cur = [None, None]
            cur[0] = new_inp("inp0", True)
            cur[1] = new_inp("inp1", True)
            for l in range(NL):   # h_{-1} = 0
                nc.sync.dma_start(seg3(cur[l])[:, :, 129:129 + D], zpad_d[:])
            outers(0, cur[0])

            GTAG = ["pf", "pi", "po", "pc"]
            for t in range(L):
                nxt = [None, None]
                nxt[0] = new_inp("inp0", t == 0) if t + 1 < L else None
                nxt[1] = new_inp("inp1", t == 0) if t + 1 < L else None

                for l in range(NL):
                    inp = cur[l]
                    i4 = seg4(inp)
                    # --- gate pre-activations: 4 gates x 4 dh band matmuls ---
                    ps = [psump.tile([D, CH * D], f32, tag=GTAG[g], name=GTAG[g]) for g in range(4)]
                    for g in range(4):
                        for half in range(2):
                            for dh in range(4):
                                idx = (l * 4 + g) * 4 + dh
                                rhs = i4[:, half * 4:(half + 1) * 4,
                                         dh // 2: dh // 2 + D, dh % 2]
                                nc.tensor.matmul(
                                    ps[g][:, half * 512:(half + 1) * 512],
                                    stT[:, idx * D:(idx + 1) * D],
                                    rhs,
                                    start=(dh == 0), stop=(dh == 3),
                                )
                    # --- activations (bias folded in) ---
                    fg = gatep.tile([D, CH * D], f32, tag="fg")
                    ig = gatep.tile([D, CH * D], f32, tag="ig")
                    og = gatep.tile([D, CH * D], f32, tag="og")
                    cs = gatep.tile([D, CH * D], f32, tag="cs")
                    for g, dst in enumerate((fg, ig, og)):
                        nc.scalar.activation(dst[:], ps[g][:], AF.Sigmoid,
                                             bias=bias[:, l * 4 + g: l * 4 + g + 1])
                    nc.scalar.activation(cs[:], ps[3][:], AF.Tanh,
                                         bias=bias[:, l * 4 + 3: l * 4 + 4])
                    # --- cell update ---
                    t1 = gatep.tile([D, CH * D], f32, tag="t1")
                    t2 = gatep.tile([D, CH * D], f32, tag="t2")
                    nc.vector.tensor_mul(t1[:], fg[:], c_l[l][:])
                    nc.vector.tensor_mul(t2[:], ig[:], cs[:])
                    nc.vector.tensor_add(c_l[l][:], t1[:], t2[:])
                    th = gatep.tile([D, CH * D], f32, tag="th")
                    nc.scalar.activation(th[:], c_l[l][:], AF.Tanh)
                    # --- h = og * tanh(c): route to consumers ---
                    if l == 0:
                        # h1_t -> x-part of layer-2 input (this step)
                        nc.vector.tensor_mul(seg3(cur[1])[:, :, 1:1 + D], og[:], th[:])
                        if nxt[0] is not None:
                            # copy h1_t -> h-part of next layer-1 input
                            nc.gpsimd.tensor_copy(
                                seg3(nxt[0])[:, :, 129:129 + D],
                                seg3(cur[1])[:, :, 1:1 + D],
                            )
                    else:
                        if nxt[1] is not None:
                            h2dst = seg3(nxt[1])[:, :, 129:129 + D]
                            nc.vector.tensor_mul(h2dst, og[:], th[:])
                            nc.vector.tensor_tensor(mp[:], mp[:], h2dst, op=ALU.max)
                        else:
                            h2 = gatep.tile([D, CH * D], f32, tag="h2last")
                            nc.vector.tensor_mul(h2[:], og[:], th[:])
                            nc.vector.tensor_tensor(mp[:], mp[:], h2[:], op=ALU.max)

                if nxt[0] is not None:
                    outers(t + 1, nxt[0])
                cur = nxt

            nc.sync.dma_start(out_d[:], mp[:])

    nc.compile()
    return nc


def _prep_core_inputs(xe_y, st, bias_arr, core):
    """xe_y: (B, 2, L, D) sqrt-normalized embeddings (axis1: 0=q, 1=a)."""
    sl = slice(4 * core, 4 * core + 4)
    # chains: s=0..3 -> q items, s=4..7 -> a items
    ch = np.concatenate([xe_y[sl, 0], xe_y[sl, 1]], axis=0)    # (8, L, D)
    xey = np.ascontiguousarray(ch.transpose(1, 0, 2)).reshape(L, 1, CH * D)
    zpad = np.zeros((D, CH * D), np.float32)
    return {"xey": xey, "st": st, "bias": bias_arr, "zpad": zpad}


def kernel(q, a, embed, conv_w, conv_b, lin_w, lin_b):
    from concourse import bass_utils

    q = np.asarray(q); a = np.asarray(a)
    embed = np.asarray(embed, np.float32)
    conv_w = np.asarray(conv_w, np.float32)
    conv_b = np.asarray(conv_b, np.float32)
    lin_w = np.asarray(lin_w, np.float32)
    lin_b = np.asarray(lin_b, np.float32)

    # host: embedding gather + density normalization factors
    idx = np.stack([q, a], axis=1).astype(np.int64)            # (B, 2, L)
    xe = embed[idx].astype(np.float64)                         # (B, 2, L, D)
    dot = np.sum(xe * xe, axis=-1, keepdims=True) + 1e-4
    xe_y = (xe / np.sqrt(dot)).astype(np.float32)

    # host: Toeplitz band stationaries  lhsT[(l,g,dh)] = B^T,
    # B[w, w'] = W[dh, w'-w+1]  (3 diagonals)
    st = np.zeros((NL * 4 * 4, D, D), np.float32)
    for l in range(NL):
        for g in range(4):
            W = conv_w[l, g, 0, 0]                             # (4, 3)
            for dh in range(4):
                Bm = sum(W[dh, dw] * np.eye(D, k=dw - 1) for dw in range(3))
                st[(l * 4 + g) * 4 + dh] = Bm.T.astype(np.float32)
    bias_arr = np.tile(conv_b.reshape(1, -1), (D, 1)).astype(np.float32)

    if "nc" not in _CACHE:
        _CACHE["nc"] = _build_nc()
    nc = _CACHE["nc"]

    in_maps = [_prep_core_inputs(xe_y, st, bias_arr, i) for i in range(NCORES)]
    _CACHE["in_maps"] = in_maps
    res = bass_utils.run_bass_kernel_spmd(nc, in_maps, core_ids=list(range(NCORES)))

    # host: unshard + final linear + log_softmax
    q_p = np.zeros((B, D * D), np.float32)
    a_p = np.zeros((B, D * D), np.float32)
    for i in range(NCORES):
        out = res.results[i]["mp_out"]                         # (D w, CH*D)
        for s in range(CH):
            mp_T = out[:, s * D:(s + 1) * D]                   # (w, j)
            flat = np.ascontiguousarray(mp_T.T).reshape(-1)    # j-major
            if s < 4:
                q_p[4 * i + s] = flat
            else:
                a_p[4 * i + s - 4] = flat
    qa = np.concatenate([q_p, a_p], axis=1)
    score = qa @ lin_w.T + lin_b
    m = score.max(axis=1, keepdims=True)
    ls = score - m
    lse = np.log(np.exp(ls).sum(axis=1, keepdims=True))
    return (ls - lse).astype(np.float32)



# revision 19
# speedup vs baseline: 4.2762x; 2.3017x over previous
"""Trainium2 Bass kernel for NnqlmCnnBasedLstm.

Math (per batch item, per input sequence q/a):
  xe = embed[idx]                      (L, D)       D = 128
  dens_t = outer(xe_t, xe_t)/(|xe_t|^2 + 1e-4)     (D, D), symmetric
  2-layer ConvLSTM over L=40 steps; each gate g:
    pre_g = conv2d([xt; h], W_g, stride=(2,1), pad=(1,1)) + b_g  on (2D, D) -> (D, D)
  c = sig(f)*c + sig(i)*tanh(cc); h = sig(o)*tanh(c)
  out = max_t h2_t  -> flatten -> concat(q,a) -> linear(2) -> log_softmax

Device strategy (8 cores, data parallel over B=32 -> 4 items/core, each with a
q-chain and an a-chain = 8 chains/core):
  * State kept TRANSPOSED: tiles are (w partitions, j free); densities are
    symmetric so layer-1 inputs need no transpose.
  * conv: out_T[w, j] = sum_{dh,dw} W[dh,dw] * inp_T[w-1+dw, 2j-1+dh].
    For each dh this is a 3-diagonal Toeplitz band matrix (over w) applied on
    the TensorEngine, with the (2j-1+dh) selection expressed as a stride-2
    free-axis access pattern on the moving operand.  All matmul operands are
    bf16 (1 cycle/col and fast weight load vs 4 cycles/col for fp32).
  * The 8 chains are split into two independent groups of 4 (q-items,
    a-items).  Per step the PE stream is L1G0, L1G1, L2G0, OUT_G0, L2G1,
    OUT_G1 so each group's activation/cell tail hides under the other
    group's matmuls and the PE never stalls (keeps HAM at full clock).
  * Gate issue order [o, f, i, c]; sigmoid/tanh on ScalarE reading PSUM
    (bias folded in); cell updates on VectorE in bf16 (2x DVE mode);
    h1 fan-out copy on GpSimd; density PSUM->SBUF copies on VectorE.
  * Embedding gather, final linear + log_softmax on host (tiny).
"""

import os
import sys

import numpy as np
import ml_dtypes

for _p in ("/opt/trn_rl_repo", "/root/.axon_site/_ro/trn_rl_repo"):
    if os.path.isdir(_p) and _p not in sys.path:
        sys.path.insert(0, _p)

BF16 = np.dtype(ml_dtypes.bfloat16)

B, L, D, V, NL = 32, 40, 128, 32000, 2
NCORES = 8
CH = 8            # chains per core: 4 batch items x {q, a}
GW = 4            # chains per pipeline group
SEG = 260         # per-chain span: [z z x(128) h(128) z z]
XOFF, HOFF = 2, 130
NF = CH * SEG
GF = GW * D       # free width of one group (512)

_CACHE = {}


def _build_nc(L=L):
    import concourse.bass as bass
    import concourse.bacc as bacc
    import concourse.mybir as mybir
    from concourse import tile

    f32 = mybir.dt.float32
    bf16 = mybir.dt.bfloat16
    AF = mybir.ActivationFunctionType
    ALU = mybir.AluOpType

    nc = bacc.Bacc(None, target_bir_lowering=False)

    xey_d = nc.dram_tensor("xey", (L, 1, CH * D), bf16, kind="ExternalInput")
    st_d = nc.dram_tensor("st", (NL * 4 * 4, D, D), bf16, kind="ExternalInput")
    bias_d = nc.dram_tensor("bias", (D, NL * 4), f32, kind="ExternalInput")
    zpad_d = nc.dram_tensor("zpad", (D, CH * D), bf16, kind="ExternalInput")
    out_d = nc.dram_tensor("mp_out", (D, CH * D), bf16, kind="ExternalOutput")

    # gate order per window: o first (off critical path), cc last
    GORDER = [2, 0, 1, 3]          # conv_w gate indices: f,i,o,cc -> o,f,i,cc
    GTAG = {2: "po", 0: "pf", 1: "pi", 3: "pc"}

    with tile.TileContext(nc) as tc:
        with (
            tc.tile_pool(name="const", bufs=1) as constp,
            tc.tile_pool(name="state", bufs=1) as statep,
            tc.tile_pool(name="inp", bufs=2) as inpp,
            tc.tile_pool(name="gate", bufs=2) as gatep,
            tc.tile_pool(name="stage", bufs=2) as stagep,
            tc.tile_pool(name="psum", bufs=2, space="PSUM") as psump,
        ):
            # ---- constants ----
            stT = constp.tile([D, NL * 4 * 4 * D], bf16, tag="stT")
            for i in range(NL * 4 * 4):
                nc.sync.dma_start(stT[:, i * D:(i + 1) * D], st_d[i])
            bias = constp.tile([D, NL * 4], f32, tag="bias")
            nc.sync.dma_start(bias[:], bias_d[:])

            # ---- persistent state ----
            c_l = [statep.tile([D, CH * D], bf16, tag=f"c{l}", name=f"c{l}")
                   for l in range(NL)]
            mp = statep.tile([D, CH * D], bf16, tag="mp")
            for l in range(NL):
                nc.vector.memset(c_l[l][:], 0.0)
            nc.vector.memset(mp[:], -1e30)

            def seg3(t):
                return t[:].rearrange("p (s c) -> p s c", s=CH)

            def seg4(t):
                return t[:].rearrange("p (s c two) -> p s c two", s=CH, two=2)

            def xpart(t, grp):
                return seg3(t)[:, grp * GW:(grp + 1) * GW, XOFF:XOFF + D]

            def hpart(t, grp):
                return seg3(t)[:, grp * GW:(grp + 1) * GW, HOFF:HOFF + D]

            def new_inp(tag, zero_pads):
                t = inpp.tile([D, NF], bf16, tag=tag, name=tag)
                if zero_pads:
                    # pad cols (0,1,258,259 of each chain seg) stay zero for
                    # the whole run; only each buffer's first use zeroes them
                    v = seg3(t)
                    nc.sync.dma_start(v[:, :, 0:2], zpad_d[:, 0:2 * CH])
                    nc.sync.dma_start(v[:, :, SEG - 2:SEG], zpad_d[:, 0:2 * CH])
                return t

            def stage_dma(t):
                s = stagep.tile([1, CH * D], bf16, tag="stage", name="stage")
                nc.sync.dma_start(s[:], xey_d[t])
                return s

            def outers(stg, grp):
                """Rank-1 density matmuls for one group -> psum tile."""
                po = psump.tile([D, GF], f32, tag="pf", name="pdens")
                for s in range(GW):
                    ch = grp * GW + s
                    vec = stg[0:1, ch * D:(ch + 1) * D]
                    nc.tensor.matmul(po[:, s * D:(s + 1) * D], vec, vec,
                                     start=True, stop=True)
                return po

            def band_mms(l, grp, inp):
                """Gate pre-activation matmuls for one (layer, group)."""
                i4 = seg4(inp)
                ps = {}
                for g in GORDER:
                    p = psump.tile([D, GF], f32, tag=GTAG[g], name=GTAG[g])
                    ps[g] = p
                    for dh in range(4):
                        idx = (l * 4 + g) * 4 + dh
                        c0, par = (dh + 1) // 2, (dh + 1) % 2
                        rhs = i4[:, grp * GW:(grp + 1) * GW, c0:c0 + D, par]
                        nc.tensor.matmul(
                            p[:], stT[:, idx * D:(idx + 1) * D], rhs,
                            start=(dh == 0), stop=(dh == 3),
                        )
                return ps

            def gate_acts(l, grp, ps):
                """sigmoid/tanh for the four gates of one window."""
                gt = {}
                for g, af in ((2, AF.Sigmoid), (0, AF.Sigmoid),
                              (1, AF.Sigmoid), (3, AF.Tanh)):
                    dst = gatep.tile([D, GF], bf16, tag=f"g{g}", name=f"g{g}")
                    nc.scalar.activation(dst[:], ps[g][:], af,
                                         bias=bias[:, l * 4 + g:l * 4 + g + 1])
                    gt[g] = dst
                return gt

            def cell_mid(l, grp, gt):
                """t1,t2,c update on DVE then tanh(c) on ACT."""
                cs = c_l[l][:, grp * GF:(grp + 1) * GF]
                t1 = gatep.tile([D, GF], bf16, tag="t1")
                t2 = gatep.tile([D, GF], bf16, tag="t2")
                nc.vector.tensor_mul(t1[:], gt[0][:], cs)
                nc.vector.tensor_mul(t2[:], gt[1][:], gt[3][:])
                nc.vector.tensor_add(cs, t1[:], t2[:])
                th = gatep.tile([D, GF], bf16, tag="th")
                nc.scalar.activation(th[:], cs, AF.Tanh)
                return th

            def h_store(l, grp, gt, th, dst):
                nc.vector.tensor_mul(dst, gt[2][:], th[:])

            # ---- prologue ----
            cur = [new_inp("inp0", True), new_inp("inp1", True)]
            for l in range(NL):   # h_{-1} = 0
                nc.sync.dma_start(seg3(cur[l])[:, :, HOFF:HOFF + D], zpad_d[:])
            stg = stage_dma(0)
            pd = [outers(stg, 0), outers(stg, 1)]
            for grp in range(2):
                nc.vector.tensor_copy(xpart(cur[0], grp), pd[grp][:])

            # ---- time loop ----
            for t in range(L):
                last = t + 1 >= L
                nxt = [None, None] if last else \
                    [new_inp("inp0", t == 0), new_inp("inp1", t == 0)]
                if not last:
                    stg = stage_dma(t + 1)

                # window list: (layer, group, input tile)
                wins = [(0, 0, cur[0]), (0, 1, cur[0]),
                        (1, 0, cur[1]), (1, 1, cur[1])]
                for wi, (l, grp, inp) in enumerate(wins):
                    ps = band_mms(l, grp, inp)
                    gt = gate_acts(l, grp, ps)
                    th = cell_mid(l, grp, gt)
                    if l == 0:
                        # h1 -> x-part of this step's layer-2 input
                        h_store(l, grp, gt, th, xpart(cur[1], grp))
                        if not last:
                            nc.gpsimd.tensor_copy(hpart(nxt[0], grp),
                                                  xpart(cur[1], grp))
                    else:
                        if not last:
                            hdst = hpart(nxt[1], grp)
                        else:
                            h2 = gatep.tile([D, GF], bf16, tag="h2l")
                            hdst = h2[:]
                        h_store(l, grp, gt, th, hdst)
                        nc.vector.tensor_tensor(
                            mp[:, grp * GF:(grp + 1) * GF],
                            mp[:, grp * GF:(grp + 1) * GF], hdst, op=ALU.max)

                    # density matmuls for t+1 slotted after L2G0 / L2G1
                    if not last and l == 1:
                        po = outers(stg, grp)
                        nc.vector.tensor_copy(xpart(nxt[0], grp), po[:])

                cur = nxt

            nc.sync.dma_start(out_d[:], mp[:])

    nc.compile()
    return nc


def _prep_core_inputs(xe_y, st, bias_arr, zpad, core):
    """xe_y: (B, 2, L, D) sqrt-normalized embeddings (axis1: 0=q, 1=a)."""
    sl = slice(4 * core, 4 * core + 4)
    # chains: s=0..3 -> q items (group 0), s=4..7 -> a items (group 1)
    ch = np.concatenate([xe_y[sl, 0], xe_y[sl, 1]], axis=0)    # (8, L, D)
    xey = np.ascontiguousarray(ch.transpose(1, 0, 2)).reshape(L, 1, CH * D)
    return {"xey": xey.astype(BF16), "st": st, "bias": bias_arr, "zpad": zpad}


def kernel(q, a, embed, conv_w, conv_b, lin_w, lin_b):
    from concourse import bass_utils

    q = np.asarray(q); a = np.asarray(a)
    embed = np.asarray(embed, np.float32)
    conv_w = np.asarray(conv_w, np.float32)
    conv_b = np.asarray(conv_b, np.float32)
    lin_w = np.asarray(lin_w, np.float32)
    lin_b = np.asarray(lin_b, np.float32)

    # host: embedding gather + density normalization factors
    idx = np.stack([q, a], axis=1).astype(np.int64)            # (B, 2, L)
    xe = embed[idx].astype(np.float64)                         # (B, 2, L, D)
    dot = np.sum(xe * xe, axis=-1, keepdims=True) + 1e-4
    xe_y = (xe / np.sqrt(dot)).astype(np.float32)

    # host: Toeplitz band stationaries  lhsT[(l,g,dh)] = B^T,
    # B[w, w'] = W[dh, w'-w+1]  (3 diagonals)
    st = np.zeros((NL * 4 * 4, D, D), np.float32)
    for l in range(NL):
        for g in range(4):
            W = conv_w[l, g, 0, 0]                             # (4, 3)
            for dh in range(4):
                Bm = sum(W[dh, dw] * np.eye(D, k=dw - 1) for dw in range(3))
                st[(l * 4 + g) * 4 + dh] = Bm.T.astype(np.float32)
    st = st.astype(BF16)
    bias_arr = np.tile(conv_b.reshape(1, -1), (D, 1)).astype(np.float32)
    zpad = np.zeros((D, CH * D), BF16)

    if "nc" not in _CACHE:
        _CACHE["nc"] = _build_nc()
    nc = _CACHE["nc"]

    in_maps = [_prep_core_inputs(xe_y, st, bias_arr, zpad, i)
               for i in range(NCORES)]
    _CACHE["in_maps"] = in_maps
    res = bass_utils.run_bass_kernel_spmd(nc, in_maps, core_ids=list(range(NCORES)))

    # host: unshard + final linear + log_softmax
    q_p = np.zeros((B, D * D), np.float32)
    a_p = np.zeros((B, D * D), np.float32)
    for i in range(NCORES):
        out = np.asarray(res.results[i]["mp_out"]).astype(np.float32)
        for s in range(CH):
            mp_T = out[:, s * D:(s + 1) * D]                   # (w, j)
            flat = np.ascontiguousarray(mp_T.T).reshape(-1)    # j-major
            if s < 4:
                q_p[4 * i + s] = flat
            else:
                a_p[4 * i + s - 4] = flat
    qa = np.concatenate([q_p, a_p], axis=1)
    score = qa @ lin_w.T + lin_b
    m = score.max(axis=1, keepdims=True)
    ls = score - m
    lse = np.log(np.exp(ls).sum(axis=1, keepdims=True))
    return (ls - lse).astype(np.float32)
